# revision 5
# baseline (speedup 1.0000x reference)
"""NT-Xent / SimCLR contrastive loss on 8 Trainium2 NeuronCores.

Math (matches the jax reference):
    z = l2_normalize(concat([emb_i, emb_j]))          # [2B, D] unit rows
    sim = z @ z.T                                     # cosine similarities
    denom_r = sum_{j != r} exp(sim_rj / T)
    pos_r   = z_r . z_{(r+B) mod 2B}                  # the positive pair
    loss = mean_r( log(denom_r) - pos_r / T )

Sharding: the 2B=8192 rows are data-parallel over 8 cores. Each core
receives the full representation matrix ROTATED by -c*1024 rows, so its
slab is always local rows 0..1023 (one SPMD program for all cores).
Row sums are invariant to column order, and the positive partner of
local row r is always local row r+4096 (the roll is half the cycle).

Per core: normalize all 8192 rows in fp32, cast to bf16, DMA-xbar
transpose to a [D, 8192] layout, compute its [1024, 8192] similarity
slab with bf16 matmuls that stay in PSUM, and fuse exp(2*sim) + row
accumulation on the scalar engine (the similarity matrix never touches
HBM). The diagonal term exp(2*||z||^2) ~= e^2 is subtracted exactly.
Positives are computed separately in fp32 for accuracy. Each core
returns [128, 8] per-row partial losses; the host sums and divides.
"""

import numpy as np
from contextlib import ExitStack

import concourse.bass as bass
import concourse.tile as tile
from concourse import bacc, mybir
from concourse._compat import with_exitstack
from concourse.bass_utils import run_bass_kernel_spmd

# Problem shape (hardcoded per contract).
B = 4096
D = 256
R = 2 * B              # 8192 representation rows
N_CORES = 8
SLAB = R // N_CORES    # 1024 rows per core
INV_T = 2.0            # 1 / temperature (T = 0.5)
E2 = float(np.exp(2.0))

F32 = mybir.dt.float32
BF16 = mybir.dt.bfloat16

NT = R // 128          # 64 row tiles of 128
NG = 8                 # load groups of 8 row tiles (1 MiB per DMA)
M_TILES = SLAB // 128  # 8 output row tiles per core
NB = 4                 # big n-blocks of 2048 columns
NS = 4                 # 512-column matmuls per n-block


@with_exitstack
def _loss_kernel(ctx: ExitStack, tc: "tile.TileContext", out_ap: bass.AP, reps_ap: bass.AP):
    nc = tc.nc
    mult = mybir.AluOpType.mult
    add = mybir.AluOpType.add
    Exp = mybir.ActivationFunctionType.Exp
    Ln = mybir.ActivationFunctionType.Ln

    xpool = ctx.enter_context(tc.tile_pool(name="x", bufs=NG))
    stats = ctx.enter_context(tc.tile_pool(name="stats", bufs=NG))
    scales = ctx.enter_context(tc.tile_pool(name="scales", bufs=NG))
    zpool = ctx.enter_context(tc.tile_pool(name="z16", bufs=6))
    z32pool = ctx.enter_context(tc.tile_pool(name="z32", bufs=4))
    prodpool = ctx.enter_context(tc.tile_pool(name="prod", bufs=2))
    rtpool = ctx.enter_context(tc.tile_pool(name="repsT", bufs=16))
    epool = ctx.enter_context(tc.tile_pool(name="escratch", bufs=2))
    accpool = ctx.enter_context(tc.tile_pool(name="acc", bufs=1))
    fpool = ctx.enter_context(tc.tile_pool(name="final", bufs=1))
    psum = ctx.enter_context(tc.tile_pool(name="mm", bufs=2, space="PSUM"))

    # ---- Phase A: load + normalize + transpose --------------------------
    # xg[g]: [128, 8, 256] fp32, row tile t = g*8+tl holds rows t*128..t*128+127
    xg = []
    for g in range(NG):
        xt = xpool.tile([128, NG, D], F32, tag="x")
        src = reps_ap[g * 1024:(g + 1) * 1024, :].rearrange("(t p) d -> p t d", p=128)
        nc.sync.dma_start(xt[:], src)
        xg.append(xt)

    # repsT tiles: j in 0..15, each [128(d_lo), 4(t_local), 2(k), 128(p)] bf16
    # holding transposed normalized rows j*512 .. j*512+511.
    rts = [
        rtpool.tile([128, 4, 2, 128], BF16, tag="repsT", name=f"repsT{j}")
        for j in range(16)
    ]

    scale_g = []
    for g in range(NG):
        ssq = stats.tile([128, NG], F32, tag="ssq")
        prod = prodpool.tile([128, NG, D], F32, tag="prod")
        # squared elements for the whole group, then rowwise sums
        nc.vector.tensor_mul(prod[:], xg[g][:], xg[g][:])
        nc.vector.tensor_reduce(
            out=ssq[:], in_=prod[:], axis=mybir.AxisListType.X, op=add,
        )
        lnv = stats.tile([128, NG], F32, tag="lnv")
        sc = scales.tile([128, NG], F32, tag="scale")
        nc.scalar.activation(lnv[:], ssq[:], Ln)
        # scale = exp(-0.5 * ln(ssq)) = 1/sqrt(ssq)
        nc.scalar.activation(sc[:], lnv[:], Exp, scale=-0.5)
        scale_g.append(sc)

    for t in range(NT):
        g, tl = t // NG, t % NG
        z16 = zpool.tile([128, D], BF16, tag="z16")
        nc.vector.tensor_scalar(
            out=z16[:], in0=xg[g][:, tl, :],
            scalar1=scale_g[g][:, tl:tl + 1], scalar2=None, op0=mult,
        )
        j, tj = t // 4, t % 4
        for k in range(2):
            nc.sync.dma_start_transpose(
                out=rts[j][:, tj, k, :],
                in_=z16[:, k * 128:(k + 1) * 128],
            )

    # ---- Phase B: positives (fp32) --------------------------------------
    # local slab rows 0..1023 are tiles 0..7; partners are tiles 32..39.
    posneg = accpool.tile([128, M_TILES], F32, tag="posneg")  # holds -2*pos
    for i in range(M_TILES):
        # za = -(1/T) * z_i  (fp32), zb = z_j (fp32)
        za = z32pool.tile([128, D], F32, tag="z32")
        nc.vector.tensor_scalar(
            out=za[:], in0=xg[0][:, i, :],
            scalar1=scale_g[0][:, i:i + 1], scalar2=-INV_T, op0=mult, op1=mult,
        )
        zb = z32pool.tile([128, D], F32, tag="z32")
        nc.vector.tensor_scalar(
            out=zb[:], in0=xg[4][:, i, :],
            scalar1=scale_g[4][:, i:i + 1], scalar2=None, op0=mult,
        )
        prod = prodpool.tile([128, NG, D], F32, tag="prod")
        nc.vector.tensor_mul(prod[:, 0, :], za[:], zb[:])
        nc.vector.tensor_reduce(
            out=posneg[:, i:i + 1], in_=prod[:, 0, :],
            axis=mybir.AxisListType.X, op=add,
        )

    # ---- Phase C: similarity slab + fused exp/rowsum --------------------
    denacc = accpool.tile([128, M_TILES * NB], F32, tag="denacc")
    for m in range(M_TILES):
        jm, tm = m // 4, m % 4
        for nb in range(NB):
            pt = psum.tile([128, 2048], F32, tag="mm")
            for ns in range(NS):
                j = nb * NS + ns
                for k in range(2):
                    nc.tensor.matmul(
                        pt[:, ns * 512:(ns + 1) * 512],
                        lhsT=rts[jm][:, tm, k, :],
                        rhs=rts[j][:, :, k, :],
                        start=(k == 0),
                        stop=(k == 1),
                    )
            esc = epool.tile([128, 2048], BF16, tag="esc")
            nc.scalar.activation(
                esc[:], pt[:], Exp, scale=INV_T,
                accum_out=denacc[:, m * NB + nb:m * NB + nb + 1],
            )

    # ---- Phase D: finalize ----------------------------------------------
    drow = fpool.tile([128, M_TILES], F32, tag="drow")
    nc.vector.tensor_reduce(
        out=drow[:],
        in_=denacc[:].rearrange("p (m n) -> p m n", n=NB),
        axis=mybir.AxisListType.X,
        op=add,
    )
    # subtract the diagonal exp((1/T) * ||z||^2) = e^2 exactly
    dcorr = fpool.tile([128, M_TILES], F32, tag="dcorr")
    nc.vector.tensor_scalar(
        out=dcorr[:], in0=drow[:], scalar1=-E2, scalar2=None, op0=add,
    )
    ld = fpool.tile([128, M_TILES], F32, tag="ld")
    nc.scalar.activation(ld[:], dcorr[:], Ln)
    loss = fpool.tile([128, M_TILES], F32, tag="loss")
    nc.vector.tensor_add(loss[:], ld[:], posneg[:])
    nc.sync.dma_start(out_ap[:], loss[:])


_CACHE = {}


def _get_compiled():
    if "nc" not in _CACHE:
        nc = bacc.Bacc("TRN2", target_bir_lowering=False, debug=False)
        reps_in = nc.dram_tensor("reps", [R, D], F32, kind="ExternalInput")
        part_out = nc.dram_tensor("partial", [128, M_TILES], F32, kind="ExternalOutput")
        with tile.TileContext(nc) as tc:
            _loss_kernel(tc, part_out.ap(), reps_in.ap())
        nc.compile()
        _CACHE["nc"] = nc
    return _CACHE["nc"]


def make_in_maps(emb_i: np.ndarray, emb_j: np.ndarray):
    reps = np.concatenate(
        [np.asarray(emb_i, dtype=np.float32), np.asarray(emb_j, dtype=np.float32)],
        axis=0,
    )
    return [
        {"reps": np.ascontiguousarray(np.roll(reps, -c * SLAB, axis=0))}
        for c in range(N_CORES)
    ]


def run_spmd(emb_i, emb_j, **kwargs):
    nc = _get_compiled()
    in_maps = make_in_maps(emb_i, emb_j)
    return run_bass_kernel_spmd(nc, in_maps, core_ids=list(range(N_CORES)), **kwargs)


def kernel(emb_i: np.ndarray, emb_j: np.ndarray) -> np.ndarray:
    res = run_spmd(emb_i, emb_j)
    total = 0.0
    for c in range(N_CORES):
        total += float(np.sum(res.results[c]["partial"].astype(np.float64)))
    return np.array(total / R, dtype=np.float32)
